# revision 12
# baseline (speedup 1.0000x reference)
"""Trainium2 Bass kernel for nn_ConsistencyLoss (BCE + dilated-stencil consistency loss).

loss = mean( unfolded_weights * thred + bce )
  bce      = -(y_true*max(log(y_pred),-100) + (1-y_true)*max(log1p(-y_pred),-100))
  unfolded = max over 8 dilated (DIL=2) neighbors nb of |y_pred - nb|, zero-padded
  thred    = y_pred * (y_pred >= 0.5)

Strategy (8 NeuronCores, data-parallel over batch, 2 images/core):
  - Chunk tiles [128, 4096] = 2 bands x 2 images, blocks [i0b0|i0b1|i1b0|i1b1].
  - unfolded = max(c - nmin, nmax - c); nmax/nmin separable over the dilated
    3x3 window INCLUDING the center (including the center never changes the
    result since |c-c| = 0 <= unfolded).
  - Vertical (partition) shifts via SBUF->SBUF DMA; horizontal shifts via
    free-dim slices of zero-padded tiles. Stencil in bf16 on DVE (2x mode).
  - BCE logs on ScalarE: ln(x + FLT_MIN) reproduces torch's -100 clamp for
    uniform inputs (only x == 0 clamps; contribution ~1e-6 relative).
  - Product-sums (U*R, U*m, yt*lp, yt*l1p) via TensorE diagonal matmuls
    accumulated in PSUM, rhs blocks interleaved 128-col for contiguity;
    sum(l1p) rides the ACT pass accum_out. Host assembles the scalar.
"""

from contextlib import ExitStack

import numpy as np

import concourse.bacc as bacc
import concourse.tile as tile
from concourse import mybir
from concourse.bass_utils import run_bass_kernel_spmd

F32 = mybir.dt.float32
BF16 = mybir.dt.bfloat16
OP = mybir.AluOpType
AT = mybir.ActivationFunctionType

B, H, W = 16, 1024, 1024
NCORES = 8
IPC = B // NCORES          # images per core = 2
P = 128
NB = 2                     # bands per image per chunk tile
NBLK = IPC * NB            # 4 column blocks per chunk tile
NCHUNK = H // (P * NB)     # 4 chunk iterations
FW = NBLK * W              # 4096
BW = W + 4                 # padded block width
DIL = 2
TINY = 1.18e-38            # min normal fp32; ln(x+TINY) == ln(x) for x >= 2^-24

N_OUT = 2 * 256 + NCHUNK


def blkc(q):
    """column range of block q"""
    return q * W, (q + 1) * W


def _kernel_body(ctx, tc, yp, yt, out):
    nc = tc.nc

    xpool = ctx.enter_context(tc.tile_pool(name="xpool", bufs=2))
    xbpool = ctx.enter_context(tc.tile_pool(name="xbpool", bufs=3))
    ytpool = ctx.enter_context(tc.tile_pool(name="ytpool", bufs=2))
    fpool = ctx.enter_context(tc.tile_pool(name="fpool", bufs=1))    # lpl1p / rm
    shpool = ctx.enter_context(tc.tile_pool(name="shpool", bufs=1))  # xu/xd
    vpool = ctx.enter_context(tc.tile_pool(name="vpool", bufs=1))    # padded vmax/vmin
    spool = ctx.enter_context(tc.tile_pool(name="spool", bufs=1))    # stencil temps
    upool = ctx.enter_context(tc.tile_pool(name="upool", bufs=2))
    single = ctx.enter_context(tc.tile_pool(name="single", bufs=1))
    psum = ctx.enter_context(tc.tile_pool(name="psum", bufs=1, space="PSUM"))

    l1pacc = single.tile([P, NCHUNK], F32)
    psum_a = psum.tile([P, 256], F32)
    psum_b = psum.tile([P, 256], F32)

    bias_tiny = single.tile([P, 1], F32)
    nc.gpsimd.memset(bias_tiny, TINY)
    bias_one = single.tile([P, 1], F32)
    nc.gpsimd.memset(bias_one, 1.0)
    bias_neghalf = single.tile([P, 1], F32)
    nc.gpsimd.memset(bias_neghalf, -0.5)

    zrow = single.tile([DIL, W], BF16)
    nc.vector.memset(zrow, 0.0)

    xb_tiles = {}
    rm_tiles = {}

    n_pieces = FW // P  # 32 lhsT pieces per chunk per stream

    def band_rows(c, s):
        r0 = (c * NB + s) * P
        return r0, r0 + P

    def load_chunk(c):
        x = xpool.tile([P, FW], F32, name=f"x_{c}", tag="x")
        xb = xbpool.tile([P, FW], BF16, name=f"xb_{c}", tag="xb")
        ytb = ytpool.tile([P, FW], BF16, name=f"ytb_{c}", tag="ytb")
        for img in range(IPC):
            for s in range(NB):
                q = img * NB + s
                c0, c1 = blkc(q)
                r0, r1 = band_rows(c, s)
                nc.sync.dma_start(out=x[:, c0:c1], in_=yp[img, r0:r1, :])
                # casting loads (SWDGE)
                nc.gpsimd.dma_start(out=xb[:, c0:c1], in_=yp[img, r0:r1, :])
                nc.gpsimd.dma_start(out=ytb[:, c0:c1], in_=yt[img, r0:r1, :])
        xb_tiles[c] = xb

        # [lp|l1p] interleaved at 128 cols: piece j occupies cols [256j, 256j+256)
        lpl1p = fpool.tile([P, 2 * FW], BF16, name=f"lpl1p_{c}", tag="lpl1p")
        lp4 = lpl1p.rearrange("p (j t w) -> p j t w", t=2, w=P)
        nc.scalar.activation(lp4[:, :, 0, :], x, AT.Ln, bias=bias_tiny, scale=1.0)
        nc.scalar.activation(
            lp4[:, :, 1, :], x, AT.Ln, bias=bias_one, scale=-1.0,
            accum_out=l1pacc[:, c:c + 1],
        )

        # [R|m] interleaved the same way; R on ACT, m on DVE
        rm = fpool.tile([P, 2 * FW], BF16, name=f"rm_{c}", tag="rm")
        rm4 = rm.rearrange("p (j t w) -> p j t w", t=2, w=P)
        nc.scalar.activation(rm4[:, :, 0, :], x, AT.Relu, bias=bias_neghalf, scale=1.0)
        nc.vector.tensor_scalar(
            out=rm4[:, :, 1, :], in0=x, scalar1=0.5, scalar2=None, op0=OP.is_ge,
        )
        rm_tiles[c] = rm

        # BCE product-sums: psum_b[m, :] += sum_k ytb[k, 128j+m] * [lp|l1p](j)[k, :]
        for j in range(n_pieces):
            nc.tensor.matmul(
                psum_b,
                ytb[:, j * P:(j + 1) * P],
                lpl1p[:, j * 256:(j + 1) * 256],
                start=(c == 0 and j == 0),
                stop=(c == NCHUNK - 1 and j == n_pieces - 1),
            )

    def stencil_chunk(c):
        xbc = xb_tiles[c]

        # vertical +-2 partition shifts; per-block halo fixups
        xu = shpool.tile([P, FW], BF16, name=f"xu_{c}", tag="xu")
        xd = shpool.tile([P, FW], BF16, name=f"xd_{c}", tag="xd")
        nc.sync.dma_start(out=xu[0:P - DIL, :], in_=xbc[DIL:P, :])
        nc.sync.dma_start(out=xd[DIL:P, :], in_=xbc[0:P - DIL, :])
        for img in range(IPC):
            for s in range(NB):
                q = img * NB + s
                c0, c1 = blkc(q)
                # bottom halo of block q: first rows of the next band down
                if s + 1 < NB:
                    n0, n1 = blkc(img * NB + s + 1)
                    nc.sync.dma_start(out=xu[P - DIL:P, c0:c1], in_=xbc[0:DIL, n0:n1])
                elif c + 1 < NCHUNK:
                    n0, n1 = blkc(img * NB)
                    nc.sync.dma_start(
                        out=xu[P - DIL:P, c0:c1], in_=xb_tiles[c + 1][0:DIL, n0:n1])
                else:
                    nc.sync.dma_start(out=xu[P - DIL:P, c0:c1], in_=zrow)
                # top halo of block q: last rows of the previous band up
                if s > 0:
                    n0, n1 = blkc(img * NB + s - 1)
                    nc.sync.dma_start(out=xd[0:DIL, c0:c1], in_=xbc[P - DIL:P, n0:n1])
                elif c > 0:
                    n0, n1 = blkc(img * NB + NB - 1)
                    nc.sync.dma_start(
                        out=xd[0:DIL, c0:c1], in_=xb_tiles[c - 1][P - DIL:P, n0:n1])
                else:
                    nc.sync.dma_start(out=xd[0:DIL, c0:c1], in_=zrow)

        # vertical 3-max / 3-min into zero-padded tiles
        vmax = vpool.tile([P, NBLK * BW], BF16, name=f"vmax_{c}", tag="vmax")
        vmin = vpool.tile([P, NBLK * BW], BF16, name=f"vmin_{c}", tag="vmin")
        for v in (vmax, vmin):
            for q in range(NBLK):
                nc.gpsimd.memset(v[:, q * BW:q * BW + 2], 0.0)
                nc.gpsimd.memset(v[:, q * BW + BW - 2:(q + 1) * BW], 0.0)
        vmax3 = vmax.rearrange("p (q w) -> p q w", q=NBLK)
        vmin3 = vmin.rearrange("p (q w) -> p q w", q=NBLK)

        def b3(t):
            return t.rearrange("p (q w) -> p q w", q=NBLK)

        va = spool.tile([P, FW], BF16, name=f"va_{c}", tag="g1")
        nc.vector.tensor_tensor(out=va, in0=xu, in1=xd, op=OP.max)
        nc.vector.tensor_tensor(
            out=vmax3[:, :, 2:2 + W], in0=b3(va), in1=b3(xbc), op=OP.max)
        vb = spool.tile([P, FW], BF16, name=f"vb_{c}", tag="g2")
        nc.vector.tensor_tensor(out=vb, in0=xu, in1=xd, op=OP.min)
        nc.vector.tensor_tensor(
            out=vmin3[:, :, 2:2 + W], in0=b3(vb), in1=b3(xbc), op=OP.min)

        # horizontal dilated 3-max / 3-min
        nxa = spool.tile([P, FW], BF16, name=f"nxa_{c}", tag="g1")
        nc.vector.tensor_tensor(
            out=b3(nxa), in0=vmax3[:, :, 0:W], in1=vmax3[:, :, 4:4 + W], op=OP.max)
        nx = spool.tile([P, FW], BF16, name=f"nx_{c}", tag="g3")
        nc.vector.tensor_tensor(
            out=b3(nx), in0=b3(nxa), in1=vmax3[:, :, 2:2 + W], op=OP.max)
        nma = spool.tile([P, FW], BF16, name=f"nma_{c}", tag="g2")
        nc.vector.tensor_tensor(
            out=b3(nma), in0=vmin3[:, :, 0:W], in1=vmin3[:, :, 4:4 + W], op=OP.min)
        nm = spool.tile([P, FW], BF16, name=f"nm_{c}", tag="g4")
        nc.vector.tensor_tensor(
            out=b3(nm), in0=b3(nma), in1=vmin3[:, :, 2:2 + W], op=OP.min)

        # unfolded = max(xb - nmin, nmax - xb)
        u1 = spool.tile([P, FW], BF16, name=f"u1_{c}", tag="g1")
        nc.vector.tensor_tensor(out=u1, in0=xbc, in1=nm, op=OP.subtract)
        u2 = spool.tile([P, FW], BF16, name=f"u2_{c}", tag="g2")
        nc.vector.tensor_tensor(out=u2, in0=nx, in1=xbc, op=OP.subtract)
        u = upool.tile([P, FW], BF16, name=f"u_{c}", tag="u")
        nc.vector.tensor_tensor(out=u, in0=u1, in1=u2, op=OP.max)

        # psum_a[m, :] += sum_k u[k, 128j+m] * [R|m](j)[k, :]
        rmc = rm_tiles[c]
        for j in range(n_pieces):
            nc.tensor.matmul(
                psum_a,
                u[:, j * P:(j + 1) * P],
                rmc[:, j * 256:(j + 1) * 256],
                start=(c == 0 and j == 0),
                stop=(c == NCHUNK - 1 and j == n_pieces - 1),
            )

    # software pipeline: load chunk c while running the stencil on chunk c-1
    for c in range(NCHUNK + 1):
        if c < NCHUNK:
            load_chunk(c)
        if c >= 1:
            stencil_chunk(c - 1)

    res = single.tile([P, 512], F32)
    nc.vector.tensor_copy(out=res[:, 0:256], in_=psum_a)
    nc.vector.tensor_copy(out=res[:, 256:512], in_=psum_b)
    nc.sync.dma_start(out=out[:, 0:512], in_=res)
    nc.sync.dma_start(out=out[:, 512:512 + NCHUNK], in_=l1pacc)


_CACHED = {}


def _build():
    if "nc" in _CACHED:
        return _CACHED["nc"]
    nc = bacc.Bacc(
        "TRN2",
        target_bir_lowering=False,
        debug=False,
        num_devices=NCORES,
    )
    yp = nc.dram_tensor("y_pred", [IPC, H, W], F32, kind="ExternalInput").ap()
    yt = nc.dram_tensor("y_true", [IPC, H, W], F32, kind="ExternalInput").ap()
    out = nc.dram_tensor("out", [P, N_OUT], F32, kind="ExternalOutput").ap()
    with tile.TileContext(nc) as tc:
        with ExitStack() as ctx:
            _kernel_body(ctx, tc, yp, yt, out)
    nc.compile()
    _CACHED["nc"] = nc
    return nc


def _host_reduce(outs):
    """Assemble the scalar loss from the 8 per-core [P, N_OUT] partial tensors."""
    total = np.float64(0.0)
    idx = np.arange(P)
    for o in outs:
        o = np.asarray(o, dtype=np.float64)
        a, bq, l1 = o[:, 0:256], o[:, 256:512], o[:, 512:512 + NCHUNK]
        sum_ur = a[idx, idx].sum()          # sum U * relu(x-.5)
        sum_um = a[idx, 128 + idx].sum()    # sum U * (x>=.5)
        sum_ylp = bq[idx, idx].sum()        # sum yt * ln(x)
        sum_yl1p = bq[idx, 128 + idx].sum() # sum yt * ln(1-x)
        sum_l1p = l1.sum()                  # sum ln(1-x)
        total += (sum_ur + 0.5 * sum_um) - sum_ylp - sum_l1p + sum_yl1p
    return np.float32(total / (B * H * W))


def kernel(y_true, y_pred):
    y_true = np.ascontiguousarray(np.asarray(y_true, dtype=np.float32)).reshape(B, H, W)
    y_pred = np.ascontiguousarray(np.asarray(y_pred, dtype=np.float32)).reshape(B, H, W)

    nc = _build()
    in_maps = []
    for r in range(NCORES):
        in_maps.append({
            "y_pred": np.ascontiguousarray(y_pred[r * IPC:(r + 1) * IPC]),
            "y_true": np.ascontiguousarray(y_true[r * IPC:(r + 1) * IPC]),
        })
    res = run_bass_kernel_spmd(nc, in_maps, core_ids=list(range(NCORES)))
    outs = [res.results[r]["out"] for r in range(NCORES)]
    return _host_reduce(outs)
